# revision 1
# baseline (speedup 1.0000x reference)
"""Trainium2 Bass kernel for nn_MHInrAttn (sparse_attention, b=4 s=1024 f=1024 h=16).

Strategy (8 NeuronCores):
  - The reference uses a raw .reshape(b, h, s, d_h) with NO transpose, so head h's
    Q/K/V data comes from ROWS [64h, 64h+64) of the projected [s, f] matrix.
    Sharding 2 heads per core means each core only needs 128 rows of x per batch.
  - Per core: project Q/K/V for its 128 rows (all 4 batches), run attention for its
    2 heads x 4 batches in a "transposed" orientation (scores^T [k, q]), and produce
    a partial output projection (its heads' contribution through Wo rows).
  - Host: shard inputs, run SPMD on 8 cores, sum the 8 partials, transpose, add bo.

Device-side details:
  - str_mat is host-transposed+masked (-40 fill) so it streams naturally as [k, q].
  - softmax(k-dim = partition) sums via a ones-column matmul on the PE;
    1/rowsum broadcasts across partitions via K=1 outer-product matmuls.
  - PV matmul carries an extra ones column in V to produce the second softmax's
    row sums for free; normalization is applied to the [64, 1024] PV output.
  - All matmuls run as float32r (full fp32 data, 1 cycle/row at free-dim >= 256).
"""

import numpy as np

B, S, F, H, D = 4, 1024, 1024, 16, 64
NCORES = 8
HPC = H // NCORES  # heads per core
P = 128
NEG_FILL = -40.0

_CACHE = {}


def _build_nc(mm_dt_name="float32", causal=True):
    from contextlib import ExitStack

    import concourse.bacc as bacc
    import concourse.tile as tile
    from concourse import mybir

    dt = mybir.dt
    f32 = dt.float32
    mmdt = getattr(dt, mm_dt_name)
    Exp = mybir.ActivationFunctionType.Exp

    nc = bacc.Bacc("TRN2", target_bir_lowering=False, debug=False)

    xT_d = nc.dram_tensor("xT", [B, F, P], f32, kind="ExternalInput").ap()
    str_d = nc.dram_tensor("strT", [B, HPC, S, S], f32, kind="ExternalInput").ap()
    wq_d = nc.dram_tensor("wq", [F, F], f32, kind="ExternalInput").ap()
    wk_d = nc.dram_tensor("wk", [F, F], f32, kind="ExternalInput").ap()
    wv_d = nc.dram_tensor("wv", [F, F], f32, kind="ExternalInput").ap()
    wo_d = nc.dram_tensor("wo", [P, F], f32, kind="ExternalInput").ap()
    bias_d = nc.dram_tensor("bqkv", [3, F], f32, kind="ExternalInput").ap()
    ident_d = nc.dram_tensor("ident", [P, P], f32, kind="ExternalInput").ap()
    out_d = nc.dram_tensor("outT", [B, F, S], f32, kind="ExternalOutput").ap()

    def mm(ap):
        return ap.bitcast(mmdt)

    with ExitStack() as ctx:
        tc = ctx.enter_context(tile.TileContext(nc))
        consts = ctx.enter_context(tc.tile_pool(name="consts", bufs=1))
        qtkt = ctx.enter_context(tc.tile_pool(name="qtkt", bufs=1))
        v2p = ctx.enter_context(tc.tile_pool(name="v2", bufs=1))
        outp = ctx.enter_context(tc.tile_pool(name="outp", bufs=1))
        wop = ctx.enter_context(tc.tile_pool(name="wop", bufs=1))
        dramp = ctx.enter_context(tc.tile_pool(name="dram", bufs=1, space="DRAM"))

        ident = consts.tile([P, P], f32, tag="ident", name="ident")
        nc.sync.dma_start(out=ident, in_=ident_d)
        ones_all = consts.tile([P, P], f32, tag="ones", name="ones")
        nc.vector.memset(ones_all, 1.0)
        bias_sb = consts.tile([1, 3 * F], f32, tag="bias", name="bias")
        nc.sync.dma_start(out=bias_sb, in_=bias_d.rearrange("a b -> (a b)").unsqueeze(0))
        wo_sb = wop.tile([P, F], f32, tag="wo", name="wo")
        nc.sync.dma_start(out=wo_sb, in_=wo_d)

        QT, KT, V2, OT = {}, {}, {}, {}
        for b in range(B):
            QT[b] = qtkt.tile([P, S], f32, tag=f"qt{b}", name=f"qt{b}")
            KT[b] = qtkt.tile([P, S], f32, tag=f"kt{b}", name=f"kt{b}")
            OT[b] = outp.tile([P, S], f32, tag=f"ot{b}", name=f"ot{b}")
            for hp in range(HPC):
                V2[b, hp] = v2p.tile([P, 8, P], f32, tag=f"v{b}{hp}", name=f"v{b}{hp}")

        # ---------- phase 1: projections + layout shuffles ----------
        with tc.tile_pool(name="xt", bufs=1) as xtp, \
                tc.tile_pool(name="wpool", bufs=1) as wp, \
                tc.tile_pool(name="qkvc", bufs=1) as qkvcp, \
                tc.tile_pool(name="pj", bufs=2, space="PSUM") as ppool, \
                tc.tile_pool(name="tp", bufs=2, space="PSUM") as tpool:
            xt = {}
            for b in range(B):
                xt[b] = xtp.tile([P, 8, P], f32, tag=f"xt{b}", name=f"xt{b}")
                nc.sync.dma_start(out=xt[b], in_=xT_d[b].rearrange("(kc p) r -> p kc r", p=P))

            qkvc = {}
            for t_i, w_d in enumerate([wq_d, wk_d, wv_d]):
                wt = []
                for i in range(8):
                    w_tile = wp.tile([P, F], f32, tag=f"w{i}", name=f"w{i}")
                    nc.sync.dma_start(out=w_tile, in_=w_d[i * P:(i + 1) * P, :])
                    wt.append(w_tile)
                for b in range(B):
                    cc = qkvcp.tile([P, F], f32, tag=f"c{t_i}{b}", name=f"c{t_i}{b}")
                    qkvc[t_i, b] = cc
                    for h2 in range(2):
                        ps = ppool.tile([P, 512], f32, tag="pj", name="pj")
                        for kc in range(8):
                            nc.tensor.matmul(
                                ps, mm(xt[b][:, kc, :]),
                                mm(wt[kc][:, 512 * h2:512 * (h2 + 1)]),
                                start=(kc == 0), stop=False)
                        nc.tensor.matmul(
                            ps, mm(ones_all[0:1, :]),
                            mm(bias_sb[0:1, 1024 * t_i + 512 * h2:1024 * t_i + 512 * h2 + 512]),
                            start=False, stop=True)
                        nc.scalar.copy(cc[:, 512 * h2:512 * (h2 + 1)], ps)

            # V shuffle through DRAM into [s'-chunk partitions, d] layout (+ones col)
            vs = {}
            for b in range(B):
                vs[b] = dramp.tile([P, F], f32, tag=f"vs{b}", name=f"vs{b}")
                nc.sync.dma_start(out=vs[b], in_=qkvc[2, b][:])
            for b in range(B):
                for hp in range(HPC):
                    nc.vector.memset(V2[b, hp], 0.0)
                    dcol = 64 * hp
                    ones_col = 64 if hp == 0 else 0
                    src = vs[b][64 * hp:64 * hp + 64, :].rearrange(
                        "(j r) (cb d) -> (r cb) j d", j=8, cb=16)
                    nc.sync.dma_start(out=V2[b, hp][:, :, dcol:dcol + 64], in_=src)
                    nc.vector.memset(V2[b, hp][:, :, ones_col:ones_col + 1], 1.0)

            # Q^T / K^T via 64x64 PE transposes (both heads stacked on partitions)
            for b in range(B):
                for t_i, dstmap in ((0, QT), (1, KT)):
                    for half in range(2):
                        # transpose psum outputs must be at partition 0; the
                        # DVE copy shifts head 1 back up to partitions 64-127
                        psts = []
                        for hp in range(HPC):
                            base = 64 * hp
                            pst = tpool.tile([P, 512], f32, tag=f"tp{hp}", name=f"tp{hp}")
                            psts.append(pst)
                            for cb8 in range(8):
                                cb = 8 * half + cb8
                                nc.tensor.transpose(
                                    pst[0:64, 64 * cb8:64 * cb8 + 64],
                                    qkvc[t_i, b][base:base + 64, 64 * cb:64 * cb + 64],
                                    ident[base:base + 64, base:base + 64])
                        for hp in range(HPC):
                            dst = dstmap[b][64 * hp:64 * hp + 64, :].rearrange(
                                "p (r cb) -> p cb r", cb=16)[:, 8 * half:8 * half + 8, :]
                            nc.vector.tensor_copy(
                                dst, psts[hp][0:64, :].rearrange("p (cb8 r) -> p cb8 r", cb8=8))

        # ---------- phase 2: attention ----------
        with tc.tile_pool(name="em", bufs=1) as emp, \
                tc.tile_pool(name="ep", bufs=3) as epool, \
                tc.tile_pool(name="misc", bufs=2) as miscp, \
                tc.tile_pool(name="aps", bufs=1, space="PSUM") as aps, \
                tc.tile_pool(name="qkps", bufs=2, space="PSUM") as qkps:
            for b in range(B):
                eM, r1bc = {}, {}
                for hp in range(HPC):
                    ps_r1 = [aps.tile([1, 512], f32, tag=f"r1_{h2}", name=f"r1_{h2}") for h2 in range(2)]
                    for j in range(8):
                        jl = 128 * j if causal else 0
                        w = S - jl
                        t = emp.tile([P, w], f32, tag=f"e{hp}{j}", name=f"e{hp}{j}")
                        eM[hp, j] = t
                        nc.sync.dma_start(out=t, in_=str_d[b, hp, 128 * j:128 * (j + 1), jl:])
                        nc.scalar.activation(t, t, Exp)
                        for h2 in range(2):
                            lo = max(512 * h2, jl)
                            hi = 512 * (h2 + 1)
                            if lo < hi:
                                last_j = (3 if h2 == 0 else 7) if causal else 7
                                nc.tensor.matmul(
                                    ps_r1[h2][0:1, lo - 512 * h2:hi - 512 * h2],
                                    mm(ones_all[:, 0:1]), mm(t[:, lo - jl:hi - jl]),
                                    start=(j == 0), stop=(j == last_j))
                    r1sb = miscp.tile([1, S], f32, tag=f"r1sb{hp}", name=f"r1sb{hp}")
                    rbc = miscp.tile([P, S], f32, tag=f"r1bc{hp}", name=f"r1bc{hp}")
                    r1bc[hp] = rbc
                    for h2 in range(2):
                        sl = slice(512 * h2, 512 * (h2 + 1))
                        nc.vector.reciprocal(r1sb[:, sl], ps_r1[h2])
                        psb = aps.tile([P, 512], f32, tag="bc", name="bc")
                        nc.tensor.matmul(psb, mm(ones_all[0:1, :]), mm(r1sb[0:1, sl]),
                                         start=True, stop=True)
                        nc.vector.tensor_copy(rbc[:, sl], psb)

                for hp in range(HPC):
                    base = 64 * hp
                    pv = [aps.tile([P, 512], f32, tag=f"pv{h2}", name=f"pv{h2}") for h2 in range(2)]
                    for j in range(8):
                        jl = 128 * j if causal else 0
                        Ej = epool.tile([P, S], f32, tag="E", name="E")
                        for h2 in range(2):
                            lo_h, hi_h = 512 * h2, 512 * (h2 + 1)
                            qk = qkps.tile([P, 512], f32, tag="qk", name="qk")
                            nc.tensor.matmul(
                                qk, mm(KT[b][base:base + 64, 128 * j:128 * (j + 1)]),
                                mm(QT[b][base:base + 64, lo_h:hi_h]),
                                start=True, stop=True)
                            m0_hi = min(jl, hi_h)
                            if m0_hi > lo_h:
                                nc.scalar.activation(Ej[:, lo_h:m0_hi], qk[:, 0:m0_hi - lo_h], Exp)
                            v_lo = max(jl, lo_h)
                            if v_lo < hi_h:
                                sl_E = Ej[:, v_lo:hi_h]
                                nc.vector.tensor_mul(sl_E, eM[hp, j][:, v_lo - jl:hi_h - jl],
                                                     r1bc[hp][:, v_lo:hi_h])
                                nc.vector.tensor_add(sl_E, sl_E, qk[:, v_lo - lo_h:hi_h - lo_h])
                                nc.scalar.activation(sl_E, sl_E, Exp)
                            nc.tensor.matmul(pv[h2], mm(V2[b, hp][:, j, :]), mm(Ej[:, lo_h:hi_h]),
                                             start=(j == 0), stop=(j == 7))
                    # normalize rows of PV by 1/rowsum2 (from the ones column)
                    sum_row = 64 if hp == 0 else 0
                    dlo = 64 * hp
                    r2sb = miscp.tile([P, S], f32, tag="r2sb", name="r2sb")
                    r2bc = miscp.tile([P, S], f32, tag="r2bc", name="r2bc")
                    for h2 in range(2):
                        sl = slice(512 * h2, 512 * (h2 + 1))
                        nc.vector.reciprocal(r2sb[sum_row:sum_row + 1, sl],
                                             pv[h2][sum_row:sum_row + 1, :])
                        psb = aps.tile([P, 512], f32, tag="bc", name="bc")
                        nc.tensor.matmul(psb[dlo:dlo + 64, :],
                                         mm(ones_all[sum_row:sum_row + 1, 0:64]),
                                         mm(r2sb[sum_row:sum_row + 1, sl]),
                                         start=True, stop=True)
                        nc.vector.tensor_copy(r2bc[dlo:dlo + 64, sl], psb[dlo:dlo + 64, :])
                        nc.vector.tensor_mul(OT[b][dlo:dlo + 64, sl], pv[h2][dlo:dlo + 64, :],
                                             r2bc[dlo:dlo + 64, sl])

        # ---------- phase 3: partial output projection ----------
        with tc.tile_pool(name="os", bufs=3) as osp, \
                tc.tile_pool(name="ops", bufs=2, space="PSUM") as opsum:
            for b in range(B):
                for fo in range(8):
                    ot = osp.tile([P, S], f32, tag="os", name="os")
                    for h2 in range(2):
                        ps = opsum.tile([P, 512], f32, tag="op", name="op")
                        nc.tensor.matmul(ps, mm(wo_sb[:, 128 * fo:128 * (fo + 1)]),
                                         mm(OT[b][:, 512 * h2:512 * (h2 + 1)]),
                                         start=True, stop=True)
                        nc.scalar.copy(ot[:, 512 * h2:512 * (h2 + 1)], ps)
                    nc.sync.dma_start(out=out_d[b, 128 * fo:128 * (fo + 1), :], in_=ot)

    nc.compile()
    return nc


def _prep_host(x, str_mat, attn_mask, Wq, bq, Wk, bk, Wv, bv, Wo, bo):
    x = np.asarray(x, np.float32)
    str_mat = np.asarray(str_mat, np.float32)
    attn_mask = np.asarray(attn_mask, np.float32)
    mask = attn_mask[:, 0]  # [b, s, s]
    causal = bool((mask == np.tril(np.ones((S, S), np.float32))[None]).all())
    strT = np.where(mask[:, None] == 0.0, NEG_FILL, str_mat).transpose(0, 1, 3, 2)
    xT = x.transpose(0, 2, 1)  # [b, f, s]
    Wq_s = (np.asarray(Wq, np.float32) / D)
    bq_s = (np.asarray(bq, np.float32) / D)
    bias = np.stack([bq_s, np.asarray(bk, np.float32), np.asarray(bv, np.float32)])
    ident = np.eye(P, dtype=np.float32)
    in_maps = []
    for c in range(NCORES):
        in_maps.append({
            "xT": np.ascontiguousarray(xT[:, :, P * c:P * (c + 1)]),
            "strT": np.ascontiguousarray(strT[:, HPC * c:HPC * (c + 1)]),
            "wq": Wq_s, "wk": np.asarray(Wk, np.float32), "wv": np.asarray(Wv, np.float32),
            "wo": np.ascontiguousarray(np.asarray(Wo, np.float32)[P * c:P * (c + 1)]),
            "bqkv": bias, "ident": ident,
        })
    return in_maps, causal


def kernel(**inputs):
    from concourse.bass_utils import run_bass_kernel_spmd

    in_maps, causal = _prep_host(**inputs)
    key = ("float32", causal)
    if key not in _CACHE:
        _CACHE[key] = _build_nc(mm_dt_name=key[0], causal=causal)
    nc = _CACHE[key]
    res = run_bass_kernel_spmd(nc, in_maps, core_ids=list(range(NCORES)))
    partials = [r["outT"] for r in res.results]
    out = np.sum(partials, axis=0, dtype=np.float32)  # [b, f, s]
    out = out.transpose(0, 2, 1) + np.asarray(inputs["bo"], np.float32)
    return np.ascontiguousarray(out.astype(np.float32))



# revision 11
# speedup vs baseline: 1.7246x; 1.7246x over previous
"""Trainium2 Bass kernel for nn_MHInrAttn (sparse_attention, b=4 s=1024 f=1024 h=16).

Strategy (8 NeuronCores):
  - The reference uses a raw .reshape(b, h, s, d_h) with NO transpose, so head h's
    Q/K/V data comes from ROWS [64h, 64h+64) of the projected [s, f] matrix.
    Sharding 2 heads per core means each core only needs 128 rows of x per batch.
  - Per core: project Q/K/V for its 128 rows (all 4 batches), run attention for its
    2 heads x 4 batches in a "transposed" orientation (scores^T [k, q]), and produce
    a partial output projection (its heads' contribution through Wo rows).
  - Host: shard inputs, run SPMD on 8 cores, sum the 8 partials, transpose, add bo.

Device-side details:
  - All matmul operands are bf16 (1 cycle/row on PE, fp32 PSUM accumulation);
    fp32 is 4 cycles/row and fp32r needs producer-side rounding walrus rejects.
  - str_mat is host-transposed+masked (-40 fill) + bf16-cast so it streams as [k, q].
  - softmax(k-dim = partition) sums via a ones-column matmul on the PE;
    1/rowsum (via Ln->Exp on ACT; DVE reciprocal is ~8 cyc/elem on one partition)
    broadcasts across partitions via K=1 outer-product matmuls.
  - PV matmul carries an extra ones column in V to produce the second softmax's
    row sums for free; normalization is applied to the [64, 1024] PV output.
"""

import numpy as np

B, S, F, H, D = 4, 1024, 1024, 16, 64
NCORES = 8
HPC = H // NCORES  # heads per core
P = 128
NEG_FILL = -40.0

_CACHE = {}


def _np_bf16():
    import ml_dtypes

    return ml_dtypes.bfloat16


def _build_nc(causal=True):
    from contextlib import ExitStack

    import concourse.bacc as bacc
    import concourse.tile as tile
    from concourse import mybir

    dt = mybir.dt
    f32 = dt.float32
    bf16 = dt.bfloat16
    Exp = mybir.ActivationFunctionType.Exp
    Ln = mybir.ActivationFunctionType.Ln

    nc = bacc.Bacc("TRN2", target_bir_lowering=False, debug=False)

    xT_d = nc.dram_tensor("xT", [B, F, P], bf16, kind="ExternalInput").ap()
    str_d = nc.dram_tensor("strT", [B, HPC, S, S], bf16, kind="ExternalInput").ap()
    wq_d = nc.dram_tensor("wq", [F, F], bf16, kind="ExternalInput").ap()
    wk_d = nc.dram_tensor("wk", [F, F], bf16, kind="ExternalInput").ap()
    wv_d = nc.dram_tensor("wv", [F, F], bf16, kind="ExternalInput").ap()
    wo_d = nc.dram_tensor("wo", [P, F], bf16, kind="ExternalInput").ap()
    bias_d = nc.dram_tensor("bqkv", [3, F], bf16, kind="ExternalInput").ap()
    ident_d = nc.dram_tensor("ident", [P, P], bf16, kind="ExternalInput").ap()
    out_d = nc.dram_tensor("outT", [B, F, S], f32, kind="ExternalOutput").ap()

    def reciprocal(nc_, out_bf, tmp_f32, in_):
        # 1/x = exp(-ln x) on ACT (x > 0 always here). Ln must stay fp32:
        # bf16 log values ~0..8 would round to ~3% error after exp.
        nc_.scalar.activation(tmp_f32, in_, Ln)
        nc_.scalar.activation(out_bf, tmp_f32, Exp, scale=-1.0)

    with ExitStack() as ctx:
        tc = ctx.enter_context(tile.TileContext(nc))
        consts = ctx.enter_context(tc.tile_pool(name="consts", bufs=1))
        qtkt = ctx.enter_context(tc.tile_pool(name="qtkt", bufs=1))
        v2p = ctx.enter_context(tc.tile_pool(name="v2", bufs=1))
        outp = ctx.enter_context(tc.tile_pool(name="outp", bufs=1))
        wop = ctx.enter_context(tc.tile_pool(name="wop", bufs=1))
        dramp = ctx.enter_context(tc.tile_pool(name="dram", bufs=1, space="DRAM"))

        ident = consts.tile([P, P], bf16, tag="ident", name="ident")
        nc.sync.dma_start(out=ident, in_=ident_d)
        ones_all = consts.tile([P, P], bf16, tag="ones", name="ones")
        nc.vector.memset(ones_all, 1.0)
        bias_sb = consts.tile([1, 3 * F], bf16, tag="bias", name="bias")
        nc.sync.dma_start(out=bias_sb, in_=bias_d.rearrange("a b -> (a b)").unsqueeze(0))
        wo_sb = wop.tile([P, F], bf16, tag="wo", name="wo")
        nc.sync.dma_start(out=wo_sb, in_=wo_d)

        QT, KT, V2, OT = {}, {}, {}, {}
        for b in range(B):
            QT[b] = qtkt.tile([P, S], bf16, tag=f"qt{b}", name=f"qt{b}")
            KT[b] = qtkt.tile([P, S], bf16, tag=f"kt{b}", name=f"kt{b}")
            OT[b] = outp.tile([P, S], bf16, tag=f"ot{b}", name=f"ot{b}")
            for hp in range(HPC):
                V2[b, hp] = v2p.tile([P, 8, P], bf16, tag=f"v{b}{hp}", name=f"v{b}{hp}")

        # ---------- phase 1: projections + layout shuffles ----------
        with tc.tile_pool(name="xt", bufs=1) as xtp, \
                tc.tile_pool(name="wpool", bufs=1) as wp, \
                tc.tile_pool(name="qkvc", bufs=1) as qkvcp, \
                tc.tile_pool(name="pj", bufs=2, space="PSUM") as ppool, \
                tc.tile_pool(name="tp", bufs=2, space="PSUM") as tpool:
            xt = {}
            for b in range(B):
                xt[b] = xtp.tile([P, 8, P], bf16, tag=f"xt{b}", name=f"xt{b}")
                nc.sync.dma_start(out=xt[b], in_=xT_d[b].rearrange("(kc p) r -> p kc r", p=P))

            qkvc = {}
            for t_i, w_d in enumerate([wq_d, wk_d, wv_d]):
                wt = []
                for i in range(8):
                    w_tile = wp.tile([P, F], bf16, tag=f"w{i}", name=f"w{i}")
                    nc.sync.dma_start(out=w_tile, in_=w_d[i * P:(i + 1) * P, :])
                    wt.append(w_tile)
                for b in range(B):
                    cc = qkvcp.tile([P, F], bf16, tag=f"c{t_i}{b}", name=f"c{t_i}{b}")
                    qkvc[t_i, b] = cc
                    for h2 in range(2):
                        ps = ppool.tile([P, 512], f32, tag="pj", name="pj")
                        for kc in range(8):
                            nc.tensor.matmul(
                                ps, xt[b][:, kc, :],
                                wt[kc][:, 512 * h2:512 * (h2 + 1)],
                                start=(kc == 0), stop=False)
                        nc.tensor.matmul(
                            ps, ones_all[0:1, :],
                            bias_sb[0:1, 1024 * t_i + 512 * h2:1024 * t_i + 512 * h2 + 512],
                            start=False, stop=True)
                        nc.scalar.copy(cc[:, 512 * h2:512 * (h2 + 1)], ps)

            # V shuffle through DRAM into [s'-chunk partitions, d] layout (+ones col)
            vs = {}
            for b in range(B):
                vs[b] = dramp.tile([P, F], bf16, tag=f"vs{b}", name=f"vs{b}")
                nc.sync.dma_start(out=vs[b], in_=qkvc[2, b][:])
            for b in range(B):
                for hp in range(HPC):
                    nc.vector.memset(V2[b, hp], 0.0)
                    dcol = 64 * hp
                    ones_col = 64 if hp == 0 else 0
                    src = vs[b][64 * hp:64 * hp + 64, :].rearrange(
                        "(j r) (cb d) -> (r cb) j d", j=8, cb=16)
                    nc.sync.dma_start(out=V2[b, hp][:, :, dcol:dcol + 64], in_=src)
                    nc.vector.memset(V2[b, hp][:, :, ones_col:ones_col + 1], 1.0)

            # Q^T / K^T via 64x64 PE transposes (both heads stacked on partitions)
            for b in range(B):
                for t_i, dstmap in ((0, QT), (1, KT)):
                    for half in range(2):
                        # transpose psum outputs must be at partition 0; the
                        # DVE copy shifts head 1 back up to partitions 64-127
                        psts = []
                        for hp in range(HPC):
                            base = 64 * hp
                            pst = tpool.tile([P, 512], bf16, tag=f"tp{hp}", name=f"tp{hp}")
                            psts.append(pst)
                            for cb8 in range(8):
                                cb = 8 * half + cb8
                                nc.tensor.transpose(
                                    pst[0:64, 64 * cb8:64 * cb8 + 64],
                                    qkvc[t_i, b][base:base + 64, 64 * cb:64 * cb + 64],
                                    ident[base:base + 64, base:base + 64])
                        for hp in range(HPC):
                            dst = dstmap[b][64 * hp:64 * hp + 64, :].rearrange(
                                "p (r cb) -> p cb r", cb=16)[:, 8 * half:8 * half + 8, :]
                            nc.vector.tensor_copy(
                                dst, psts[hp][0:64, :].rearrange("p (cb8 r) -> p cb8 r", cb8=8))

        # ---------- phase 2: attention ----------
        with tc.tile_pool(name="em", bufs=1) as emp, \
                tc.tile_pool(name="ep", bufs=3) as epool, \
                tc.tile_pool(name="misc", bufs=2) as miscp, \
                tc.tile_pool(name="aps", bufs=1, space="PSUM") as aps, \
                tc.tile_pool(name="qkps", bufs=2, space="PSUM") as qkps:
            for b in range(B):
                eM, r1bc = {}, {}
                for hp in range(HPC):
                    ps_r1 = [aps.tile([1, 512], f32, tag=f"r1_{h2}", name=f"r1_{h2}") for h2 in range(2)]
                    for j in range(8):
                        jl = 128 * j if causal else 0
                        w = S - jl
                        t = emp.tile([P, w], bf16, tag=f"e{hp}{j}", name=f"e{hp}{j}")
                        eM[hp, j] = t
                        nc.sync.dma_start(out=t, in_=str_d[b, hp, 128 * j:128 * (j + 1), jl:])
                        nc.scalar.activation(t, t, Exp)
                        for h2 in range(2):
                            lo = max(512 * h2, jl)
                            hi = 512 * (h2 + 1)
                            if lo < hi:
                                last_j = (3 if h2 == 0 else 7) if causal else 7
                                nc.tensor.matmul(
                                    ps_r1[h2][0:1, lo - 512 * h2:hi - 512 * h2],
                                    ones_all[:, 0:1], t[:, lo - jl:hi - jl],
                                    start=(j == 0), stop=(j == last_j))
                    r1f = miscp.tile([1, S], f32, tag=f"r1f{hp}", name=f"r1f{hp}")
                    r1sb = miscp.tile([1, S], bf16, tag=f"r1sb{hp}", name=f"r1sb{hp}")
                    rbc = miscp.tile([P, S], bf16, tag=f"r1bc{hp}", name=f"r1bc{hp}")
                    r1bc[hp] = rbc
                    for h2 in range(2):
                        sl = slice(512 * h2, 512 * (h2 + 1))
                        reciprocal(nc, r1sb[:, sl], r1f[:, sl], ps_r1[h2])
                        psb = aps.tile([P, 512], f32, tag="bc", name="bc")
                        nc.tensor.matmul(psb, ones_all[0:1, :], r1sb[0:1, sl],
                                         start=True, stop=True)
                        nc.vector.tensor_copy(rbc[:, sl], psb)

                for hp in range(HPC):
                    base = 64 * hp
                    pv = [aps.tile([P, 512], f32, tag=f"pv{h2}", name=f"pv{h2}") for h2 in range(2)]
                    for j in range(8):
                        jl = 128 * j if causal else 0
                        Ej = epool.tile([P, S], bf16, tag="E", name="E")
                        for h2 in range(2):
                            lo_h, hi_h = 512 * h2, 512 * (h2 + 1)
                            qk = qkps.tile([P, 512], f32, tag="qk", name="qk")
                            nc.tensor.matmul(
                                qk, KT[b][base:base + 64, 128 * j:128 * (j + 1)],
                                QT[b][base:base + 64, lo_h:hi_h],
                                start=True, stop=True)
                            m0_hi = min(jl, hi_h)
                            if m0_hi > lo_h:
                                nc.scalar.activation(Ej[:, lo_h:m0_hi], qk[:, 0:m0_hi - lo_h], Exp)
                            v_lo = max(jl, lo_h)
                            if v_lo < hi_h:
                                sl_E = Ej[:, v_lo:hi_h]
                                nc.vector.tensor_mul(sl_E, eM[hp, j][:, v_lo - jl:hi_h - jl],
                                                     r1bc[hp][:, v_lo:hi_h])
                                nc.vector.tensor_add(sl_E, sl_E, qk[:, v_lo - lo_h:hi_h - lo_h])
                                nc.scalar.activation(sl_E, sl_E, Exp)
                            nc.tensor.matmul(pv[h2], V2[b, hp][:, j, :], Ej[:, lo_h:hi_h],
                                             start=(j == 0), stop=(j == 7))
                    # normalize rows of PV by 1/rowsum2 (from the ones column)
                    sum_row = 64 if hp == 0 else 0
                    dlo = 64 * hp
                    r2f = miscp.tile([P, S], f32, tag="r2f", name="r2f")
                    r2sb = miscp.tile([P, S], bf16, tag="r2sb", name="r2sb")
                    r2bc = miscp.tile([P, S], bf16, tag="r2bc", name="r2bc")
                    for h2 in range(2):
                        sl = slice(512 * h2, 512 * (h2 + 1))
                        reciprocal(nc, r2sb[sum_row:sum_row + 1, sl],
                                   r2f[sum_row:sum_row + 1, sl],
                                   pv[h2][sum_row:sum_row + 1, :])
                        psb = aps.tile([P, 512], f32, tag="bc", name="bc")
                        nc.tensor.matmul(psb[dlo:dlo + 64, :],
                                         ones_all[sum_row:sum_row + 1, 0:64],
                                         r2sb[sum_row:sum_row + 1, sl],
                                         start=True, stop=True)
                        nc.vector.tensor_copy(r2bc[dlo:dlo + 64, sl], psb[dlo:dlo + 64, :])
                        nc.vector.tensor_mul(OT[b][dlo:dlo + 64, sl], pv[h2][dlo:dlo + 64, :],
                                             r2bc[dlo:dlo + 64, sl])

        # ---------- phase 3: partial output projection ----------
        with tc.tile_pool(name="os", bufs=3) as osp, \
                tc.tile_pool(name="ops", bufs=2, space="PSUM") as opsum:
            for b in range(B):
                for fo in range(8):
                    ot = osp.tile([P, S], f32, tag="os", name="os")
                    for h2 in range(2):
                        ps = opsum.tile([P, 512], f32, tag="op", name="op")
                        nc.tensor.matmul(ps, wo_sb[:, 128 * fo:128 * (fo + 1)],
                                         OT[b][:, 512 * h2:512 * (h2 + 1)],
                                         start=True, stop=True)
                        nc.scalar.copy(ot[:, 512 * h2:512 * (h2 + 1)], ps)
                    nc.sync.dma_start(out=out_d[b, 128 * fo:128 * (fo + 1), :], in_=ot)

    nc.compile()
    return nc


def _prep_host(x, str_mat, attn_mask, Wq, bq, Wk, bk, Wv, bv, Wo, bo):
    bf = _np_bf16()
    x = np.asarray(x, np.float32)
    str_mat = np.asarray(str_mat, np.float32)
    attn_mask = np.asarray(attn_mask, np.float32)
    mask = attn_mask[:, 0]  # [b, s, s]
    causal = bool((mask == np.tril(np.ones((S, S), np.float32))[None]).all())
    strT = np.where(mask[:, None] == 0.0, NEG_FILL, str_mat).transpose(0, 1, 3, 2)
    strT = strT.astype(bf)
    xT = x.transpose(0, 2, 1).astype(bf)  # [b, f, s]
    Wq_s = (np.asarray(Wq, np.float32) / D).astype(bf)
    bq_s = (np.asarray(bq, np.float32) / D)
    bias = np.stack([bq_s, np.asarray(bk, np.float32),
                     np.asarray(bv, np.float32)]).astype(bf)
    Wk_c = np.asarray(Wk, np.float32).astype(bf)
    Wv_c = np.asarray(Wv, np.float32).astype(bf)
    Wo_c = np.asarray(Wo, np.float32).astype(bf)
    ident = np.eye(P, dtype=np.float32).astype(bf)
    in_maps = []
    for c in range(NCORES):
        in_maps.append({
            "xT": np.ascontiguousarray(xT[:, :, P * c:P * (c + 1)]),
            "strT": np.ascontiguousarray(strT[:, HPC * c:HPC * (c + 1)]),
            "wq": Wq_s, "wk": Wk_c, "wv": Wv_c,
            "wo": np.ascontiguousarray(Wo_c[P * c:P * (c + 1)]),
            "bqkv": bias, "ident": ident,
        })
    return in_maps, causal


def kernel(**inputs):
    from concourse.bass_utils import run_bass_kernel_spmd

    in_maps, causal = _prep_host(**inputs)
    key = ("bf16", causal)
    if key not in _CACHE:
        _CACHE[key] = _build_nc(causal=causal)
    nc = _CACHE[key]
    res = run_bass_kernel_spmd(nc, in_maps, core_ids=list(range(NCORES)))
    partials = [r["outT"] for r in res.results]
    out = np.sum(partials, axis=0, dtype=np.float32)  # [b, f, s]
    out = out.transpose(0, 2, 1) + np.asarray(inputs["bo"], np.float32)
    return np.ascontiguousarray(out.astype(np.float32))


# revision 15
# speedup vs baseline: 1.9433x; 1.1268x over previous
"""Trainium2 Bass kernel for nn_MHInrAttn (sparse_attention, b=4 s=1024 f=1024 h=16).

Strategy (8 NeuronCores):
  - The reference uses a raw .reshape(b, h, s, d_h) with NO transpose, so head h's
    Q/K/V data comes from ROWS [64h, 64h+64) of the projected [s, f] matrix.
    Sharding 2 heads per core means each core only needs 128 rows of x per batch.
  - Per core: project Q/K/V for its 128 rows (all 4 batches), run attention for its
    2 heads x 4 batches in a "transposed" orientation (scores^T [k, q]), and produce
    a partial output projection (its heads' contribution through Wo rows).
  - Host: shard inputs, run SPMD on 8 cores, sum the 8 partials, transpose, add bo.

Device-side details:
  - All matmul operands are bf16 (1 cycle/row on PE, fp32 PSUM accumulation);
    fp32 is 4 cycles/row and fp32r needs producer-side rounding walrus rejects.
  - str_mat is host-transposed+masked (-40 fill) + bf16-cast so it streams as [k, q].
  - softmax(k-dim = partition) sums via a ones-column matmul on the PE;
    1/rowsum (via Ln->Exp on ACT; DVE reciprocal is ~8 cyc/elem on one partition)
    broadcasts across partitions via K=1 outer-product matmuls.
  - PV matmul carries an extra ones column in V to produce the second softmax's
    row sums for free; normalization is applied to the [64, 1024] PV output.
"""

import numpy as np

B, S, F, H, D = 4, 1024, 1024, 16, 64
NCORES = 8
HPC = H // NCORES  # heads per core
P = 128
NEG_FILL = -40.0

_CACHE = {}


def _np_bf16():
    import ml_dtypes

    return ml_dtypes.bfloat16


def _build_nc(causal=True):
    from contextlib import ExitStack

    import concourse.bacc as bacc
    import concourse.tile as tile
    from concourse import mybir

    dt = mybir.dt
    f32 = dt.float32
    bf16 = dt.bfloat16
    Exp = mybir.ActivationFunctionType.Exp
    Ln = mybir.ActivationFunctionType.Ln

    nc = bacc.Bacc("TRN2", target_bir_lowering=False, debug=False)

    xT_d = nc.dram_tensor("xT", [B, F, P], bf16, kind="ExternalInput").ap()
    str_d = nc.dram_tensor("strT", [B, HPC, S, S], bf16, kind="ExternalInput").ap()
    wq_d = nc.dram_tensor("wq", [F, F], bf16, kind="ExternalInput").ap()
    wk_d = nc.dram_tensor("wk", [F, F], bf16, kind="ExternalInput").ap()
    wv_d = nc.dram_tensor("wv", [F, F], bf16, kind="ExternalInput").ap()
    wo_d = nc.dram_tensor("wo", [P, F], bf16, kind="ExternalInput").ap()
    bias_d = nc.dram_tensor("bqkv", [3, F], bf16, kind="ExternalInput").ap()
    ident_d = nc.dram_tensor("ident", [P, P], bf16, kind="ExternalInput").ap()
    out_d = nc.dram_tensor("outT", [B, F, S], f32, kind="ExternalOutput").ap()

    def reciprocal(nc_, out_bf, tmp_f32, in_):
        # 1/x = exp(-ln x) on ACT (x > 0 always here). Ln must stay fp32:
        # bf16 log values ~0..8 would round to ~3% error after exp.
        nc_.scalar.activation(tmp_f32, in_, Ln)
        nc_.scalar.activation(out_bf, tmp_f32, Exp, scale=-1.0)

    with ExitStack() as ctx:
        tc = ctx.enter_context(tile.TileContext(nc))
        consts = ctx.enter_context(tc.tile_pool(name="consts", bufs=1))
        qtkt = ctx.enter_context(tc.tile_pool(name="qtkt", bufs=1))
        v2p = ctx.enter_context(tc.tile_pool(name="v2", bufs=1))
        outp = ctx.enter_context(tc.tile_pool(name="outp", bufs=1))
        wop = ctx.enter_context(tc.tile_pool(name="wop", bufs=1))
        dramp = ctx.enter_context(tc.tile_pool(name="dram", bufs=1, space="DRAM"))

        ident = consts.tile([P, P], bf16, tag="ident", name="ident")
        nc.sync.dma_start(out=ident, in_=ident_d)
        ones_all = consts.tile([P, P], bf16, tag="ones", name="ones")
        nc.vector.memset(ones_all, 1.0)
        bias_sb = consts.tile([1, 3 * F], bf16, tag="bias", name="bias")
        nc.sync.dma_start(out=bias_sb, in_=bias_d.rearrange("a b -> (a b)").unsqueeze(0))
        wo_sb = wop.tile([P, F], bf16, tag="wo", name="wo")
        nc.sync.dma_start(out=wo_sb, in_=wo_d)

        QT, KT, V2, OT = {}, {}, {}, {}
        for b in range(B):
            QT[b] = qtkt.tile([P, S], bf16, tag=f"qt{b}", name=f"qt{b}")
            KT[b] = qtkt.tile([P, S], bf16, tag=f"kt{b}", name=f"kt{b}")
            OT[b] = outp.tile([P, S], bf16, tag=f"ot{b}", name=f"ot{b}")
            for hp in range(HPC):
                V2[b, hp] = v2p.tile([P, 8, P], bf16, tag=f"v{b}{hp}", name=f"v{b}{hp}")

        # ---------- phase 1: projections + layout shuffles ----------
        with tc.tile_pool(name="xt", bufs=1) as xtp, \
                tc.tile_pool(name="wpool", bufs=1) as wp, \
                tc.tile_pool(name="qkvc", bufs=1) as qkvcp, \
                tc.tile_pool(name="pj", bufs=2, space="PSUM") as ppool, \
                tc.tile_pool(name="tp", bufs=2, space="PSUM") as tpool:
            xt = {}
            for b in range(B):
                xt[b] = xtp.tile([P, 8, P], bf16, tag=f"xt{b}", name=f"xt{b}")
                nc.sync.dma_start(out=xt[b], in_=xT_d[b].rearrange("(kc p) r -> p kc r", p=P))

            qkvc = {}
            for t_i, w_d in enumerate([wq_d, wk_d, wv_d]):
                wt = []
                for i in range(8):
                    w_tile = wp.tile([P, F], bf16, tag=f"w{i}", name=f"w{i}")
                    nc.sync.dma_start(out=w_tile, in_=w_d[i * P:(i + 1) * P, :])
                    wt.append(w_tile)
                for b in range(B):
                    cc = qkvcp.tile([P, F], bf16, tag=f"c{t_i}{b}", name=f"c{t_i}{b}")
                    qkvc[t_i, b] = cc
                    for h2 in range(2):
                        ps = ppool.tile([P, 512], f32, tag="pj", name="pj")
                        for kc in range(8):
                            nc.tensor.matmul(
                                ps, xt[b][:, kc, :],
                                wt[kc][:, 512 * h2:512 * (h2 + 1)],
                                start=(kc == 0), stop=False)
                        nc.tensor.matmul(
                            ps, ones_all[0:1, :],
                            bias_sb[0:1, 1024 * t_i + 512 * h2:1024 * t_i + 512 * h2 + 512],
                            start=False, stop=True)
                        nc.scalar.copy(cc[:, 512 * h2:512 * (h2 + 1)], ps)

            # V shuffle through DRAM into [s'-chunk partitions, d] layout (+ones col)
            vs = {}
            for b in range(B):
                vs[b] = dramp.tile([P, F], bf16, tag=f"vs{b}", name=f"vs{b}")
                nc.sync.dma_start(out=vs[b], in_=qkvc[2, b][:])
            for b in range(B):
                for hp in range(HPC):
                    nc.vector.memset(V2[b, hp], 0.0)
                    dcol = 64 * hp
                    ones_col = 64 if hp == 0 else 0
                    src = vs[b][64 * hp:64 * hp + 64, :].rearrange(
                        "(j r) (cb d) -> (r cb) j d", j=8, cb=16)
                    nc.sync.dma_start(out=V2[b, hp][:, :, dcol:dcol + 64], in_=src)
                    nc.vector.memset(V2[b, hp][:, :, ones_col:ones_col + 1], 1.0)

            # Q^T / K^T via 64x64 PE transposes (both heads stacked on partitions)
            for b in range(B):
                for t_i, dstmap in ((0, QT), (1, KT)):
                    for half in range(2):
                        # transpose psum outputs must be at partition 0; the
                        # DVE copy shifts head 1 back up to partitions 64-127
                        psts = []
                        for hp in range(HPC):
                            base = 64 * hp
                            pst = tpool.tile([P, 512], bf16, tag=f"tp{hp}", name=f"tp{hp}")
                            psts.append(pst)
                            for cb8 in range(8):
                                cb = 8 * half + cb8
                                nc.tensor.transpose(
                                    pst[0:64, 64 * cb8:64 * cb8 + 64],
                                    qkvc[t_i, b][base:base + 64, 64 * cb:64 * cb + 64],
                                    ident[base:base + 64, base:base + 64])
                        for hp in range(HPC):
                            # inner dim contiguous on dst (cb runs) is ~4x
                            # faster on DVE than inner-strided (measured)
                            dst = dstmap[b][64 * hp:64 * hp + 64, :].rearrange(
                                "p (r cb) -> p r cb", cb=16)[:, :, 8 * half:8 * half + 8]
                            nc.vector.tensor_copy(
                                dst, psts[hp][0:64, :].rearrange("p (cb8 r) -> p r cb8", cb8=8))

        # ---------- phase 2: attention ----------
        with tc.tile_pool(name="em", bufs=1) as emp, \
                tc.tile_pool(name="ep", bufs=3) as epool, \
                tc.tile_pool(name="misc", bufs=2) as miscp, \
                tc.tile_pool(name="aps", bufs=1, space="PSUM") as aps, \
                tc.tile_pool(name="qkps", bufs=2, space="PSUM") as qkps:
            for b in range(B):
                eM, r1bc = {}, {}
                for hp in range(HPC):
                    ps_r1 = [aps.tile([1, 512], f32, tag=f"r1_{h2}", name=f"r1_{h2}") for h2 in range(2)]
                    for j in range(8):
                        jl = 128 * j if causal else 0
                        w = S - jl
                        t = emp.tile([P, w], bf16, tag=f"e{hp}{j}", name=f"e{hp}{j}")
                        eM[hp, j] = t
                        nc.sync.dma_start(out=t, in_=str_d[b, hp, 128 * j:128 * (j + 1), jl:])
                        nc.scalar.activation(t, t, Exp)
                        for h2 in range(2):
                            lo = max(512 * h2, jl)
                            hi = 512 * (h2 + 1)
                            if lo < hi:
                                last_j = (3 if h2 == 0 else 7) if causal else 7
                                nc.tensor.matmul(
                                    ps_r1[h2][0:1, lo - 512 * h2:hi - 512 * h2],
                                    ones_all[:, 0:1], t[:, lo - jl:hi - jl],
                                    start=(j == 0), stop=(j == last_j))
                    r1f = miscp.tile([1, S], f32, tag=f"r1f{hp}", name=f"r1f{hp}")
                    r1sb = miscp.tile([1, S], bf16, tag=f"r1sb{hp}", name=f"r1sb{hp}")
                    rbc = miscp.tile([P, S], bf16, tag=f"r1bc{hp}", name=f"r1bc{hp}")
                    r1bc[hp] = rbc
                    # batch the Lns then the Exps: Ln and Exp live in different
                    # ACT table sets, so interleaving them reloads the table
                    # (~1.3us) four times per hp instead of twice
                    for h2 in range(2):
                        sl = slice(512 * h2, 512 * (h2 + 1))
                        nc.scalar.activation(r1f[:, sl], ps_r1[h2], Ln)
                    for h2 in range(2):
                        sl = slice(512 * h2, 512 * (h2 + 1))
                        nc.scalar.activation(r1sb[:, sl], r1f[:, sl], Exp, scale=-1.0)
                    for h2 in range(2):
                        sl = slice(512 * h2, 512 * (h2 + 1))
                        psb = aps.tile([P, 512], f32, tag="bc", name="bc")
                        nc.tensor.matmul(psb, ones_all[0:1, :], r1sb[0:1, sl],
                                         start=True, stop=True)
                        nc.vector.tensor_copy(rbc[:, sl], psb)

                for hp in range(HPC):
                    base = 64 * hp
                    pv = [aps.tile([P, 512], f32, tag=f"pv{h2}", name=f"pv{h2}") for h2 in range(2)]
                    for j in range(8):
                        jl = 128 * j if causal else 0
                        Ej = epool.tile([P, S], bf16, tag="E", name="E")
                        for h2 in range(2):
                            lo_h, hi_h = 512 * h2, 512 * (h2 + 1)
                            qk = qkps.tile([P, 512], f32, tag="qk", name="qk")
                            nc.tensor.matmul(
                                qk, KT[b][base:base + 64, 128 * j:128 * (j + 1)],
                                QT[b][base:base + 64, lo_h:hi_h],
                                start=True, stop=True)
                            m0_hi = min(jl, hi_h)
                            if m0_hi > lo_h:
                                nc.scalar.activation(Ej[:, lo_h:m0_hi], qk[:, 0:m0_hi - lo_h], Exp)
                            v_lo = max(jl, lo_h)
                            if v_lo < hi_h:
                                sl_E = Ej[:, v_lo:hi_h]
                                nc.vector.tensor_mul(sl_E, eM[hp, j][:, v_lo - jl:hi_h - jl],
                                                     r1bc[hp][:, v_lo:hi_h])
                                nc.vector.tensor_add(sl_E, sl_E, qk[:, v_lo - lo_h:hi_h - lo_h])
                                nc.scalar.activation(sl_E, sl_E, Exp)
                            nc.tensor.matmul(pv[h2], V2[b, hp][:, j, :], Ej[:, lo_h:hi_h],
                                             start=(j == 0), stop=(j == 7))
                    # normalize rows of PV by 1/rowsum2 (from the ones column)
                    sum_row = 64 if hp == 0 else 0
                    dlo = 64 * hp
                    r2f = miscp.tile([P, S], f32, tag="r2f", name="r2f")
                    r2sb = miscp.tile([P, S], bf16, tag="r2sb", name="r2sb")
                    r2bc = miscp.tile([P, S], bf16, tag="r2bc", name="r2bc")
                    for h2 in range(2):
                        sl = slice(512 * h2, 512 * (h2 + 1))
                        nc.scalar.activation(r2f[sum_row:sum_row + 1, sl],
                                             pv[h2][sum_row:sum_row + 1, :], Ln)
                    for h2 in range(2):
                        sl = slice(512 * h2, 512 * (h2 + 1))
                        nc.scalar.activation(r2sb[sum_row:sum_row + 1, sl],
                                             r2f[sum_row:sum_row + 1, sl], Exp, scale=-1.0)
                    for h2 in range(2):
                        sl = slice(512 * h2, 512 * (h2 + 1))
                        psb = aps.tile([P, 512], f32, tag="bc", name="bc")
                        nc.tensor.matmul(psb[dlo:dlo + 64, :],
                                         ones_all[sum_row:sum_row + 1, 0:64],
                                         r2sb[sum_row:sum_row + 1, sl],
                                         start=True, stop=True)
                        nc.vector.tensor_copy(r2bc[dlo:dlo + 64, sl], psb[dlo:dlo + 64, :])
                        nc.vector.tensor_mul(OT[b][dlo:dlo + 64, sl], pv[h2][dlo:dlo + 64, :],
                                             r2bc[dlo:dlo + 64, sl])

        # ---------- phase 3: partial output projection ----------
        with tc.tile_pool(name="os", bufs=3) as osp, \
                tc.tile_pool(name="ops", bufs=2, space="PSUM") as opsum:
            for b in range(B):
                for fo in range(8):
                    ot = osp.tile([P, S], f32, tag="os", name="os")
                    for h2 in range(2):
                        ps = opsum.tile([P, 512], f32, tag="op", name="op")
                        nc.tensor.matmul(ps, wo_sb[:, 128 * fo:128 * (fo + 1)],
                                         OT[b][:, 512 * h2:512 * (h2 + 1)],
                                         start=True, stop=True)
                        nc.scalar.copy(ot[:, 512 * h2:512 * (h2 + 1)], ps)
                    nc.sync.dma_start(out=out_d[b, 128 * fo:128 * (fo + 1), :], in_=ot)

    nc.compile()
    return nc


def _prep_host(x, str_mat, attn_mask, Wq, bq, Wk, bk, Wv, bv, Wo, bo):
    bf = _np_bf16()
    x = np.asarray(x, np.float32)
    str_mat = np.asarray(str_mat, np.float32)
    attn_mask = np.asarray(attn_mask, np.float32)
    mask = attn_mask[:, 0]  # [b, s, s]
    causal = bool((mask == np.tril(np.ones((S, S), np.float32))[None]).all())
    strT = np.where(mask[:, None] == 0.0, NEG_FILL, str_mat).transpose(0, 1, 3, 2)
    strT = strT.astype(bf)
    xT = x.transpose(0, 2, 1).astype(bf)  # [b, f, s]
    Wq_s = (np.asarray(Wq, np.float32) / D).astype(bf)
    bq_s = (np.asarray(bq, np.float32) / D)
    bias = np.stack([bq_s, np.asarray(bk, np.float32),
                     np.asarray(bv, np.float32)]).astype(bf)
    Wk_c = np.asarray(Wk, np.float32).astype(bf)
    Wv_c = np.asarray(Wv, np.float32).astype(bf)
    Wo_c = np.asarray(Wo, np.float32).astype(bf)
    ident = np.eye(P, dtype=np.float32).astype(bf)
    in_maps = []
    for c in range(NCORES):
        in_maps.append({
            "xT": np.ascontiguousarray(xT[:, :, P * c:P * (c + 1)]),
            "strT": np.ascontiguousarray(strT[:, HPC * c:HPC * (c + 1)]),
            "wq": Wq_s, "wk": Wk_c, "wv": Wv_c,
            "wo": np.ascontiguousarray(Wo_c[P * c:P * (c + 1)]),
            "bqkv": bias, "ident": ident,
        })
    return in_maps, causal


def kernel(**inputs):
    from concourse.bass_utils import run_bass_kernel_spmd

    in_maps, causal = _prep_host(**inputs)
    key = ("bf16", causal)
    if key not in _CACHE:
        _CACHE[key] = _build_nc(causal=causal)
    nc = _CACHE[key]
    res = run_bass_kernel_spmd(nc, in_maps, core_ids=list(range(NCORES)))
    partials = [r["outT"] for r in res.results]
    out = np.sum(partials, axis=0, dtype=np.float32)  # [b, f, s]
    out = out.transpose(0, 2, 1) + np.asarray(inputs["bo"], np.float32)
    return np.ascontiguousarray(out.astype(np.float32))


# revision 17
# speedup vs baseline: 2.0157x; 1.0373x over previous
"""Trainium2 Bass kernel for nn_MHInrAttn (sparse_attention, b=4 s=1024 f=1024 h=16).

v2 sharding (8 NeuronCores): core c -> (batch c//2, head-group c%2).
Each core: 8 heads of one batch. The reference uses a raw .reshape with NO
transpose, so head h's Q/K/V come from rows [64h, 64h+64) of the projected
[s, f] matrix -> a head-group needs only x rows [512g, 512g+512).

Per core: project Q/K/V for 512 rows, attention for 8 heads in scores^T
[k, q] orientation, partial output projection through its 512 Wo rows.
Host: sum partial pairs per batch, transpose, add bo.

Device details:
  - all matmul operands bf16 (1 cyc/row on PE), fp32 PSUM accumulation
  - str^T streams causally trimmed into a PACKED eM tile (one wide exp/head)
  - softmax-1 k-sums via ones-column PE matmul; 1/x via Ln->Exp on ACT with
    Ln/Exp batched per phase (they live in different ACT table sets)
  - qk pair tile [128, 1024] spans 2 PSUM banks -> wide ACT/DVE ops
  - PV carries a ones column in V for softmax-2 row sums for free
"""

import numpy as np

B, S, F, H, D = 4, 1024, 1024, 16, 64
NCORES = 8
HG = 8           # heads per core (head-group)
RW = 512         # proj rows per core
P = 128
NEG_FILL = -40.0

_CACHE = {}

_WOFF = [0, 1024, 1920, 2688, 3328, 3840, 4224, 4480]  # packed eM offsets
_WTOT = 4608


def _np_bf16():
    import ml_dtypes

    return ml_dtypes.bfloat16


def _build_nc(causal=True):
    from contextlib import ExitStack

    import concourse.bacc as bacc
    import concourse.tile as tile
    from concourse import mybir

    dt = mybir.dt
    f32 = dt.float32
    bf16 = dt.bfloat16
    Exp = mybir.ActivationFunctionType.Exp
    Ln = mybir.ActivationFunctionType.Ln

    nc = bacc.Bacc("TRN2", target_bir_lowering=False, debug=False)

    xT_d = nc.dram_tensor("xT", [F, RW], bf16, kind="ExternalInput").ap()
    str_d = nc.dram_tensor("strT", [HG, S, S], bf16, kind="ExternalInput").ap()
    wq_d = nc.dram_tensor("wq", [F, F], bf16, kind="ExternalInput").ap()
    wk_d = nc.dram_tensor("wk", [F, F], bf16, kind="ExternalInput").ap()
    wv_d = nc.dram_tensor("wv", [F, F], bf16, kind="ExternalInput").ap()
    wo_d = nc.dram_tensor("wo", [RW, F], bf16, kind="ExternalInput").ap()
    bias_d = nc.dram_tensor("bqkv", [3, F], bf16, kind="ExternalInput").ap()
    ident_d = nc.dram_tensor("ident", [P, P], bf16, kind="ExternalInput").ap()
    out_d = nc.dram_tensor("outT", [F, S], f32, kind="ExternalOutput").ap()

    woff = _WOFF if causal else [1024 * j for j in range(8)]
    wtot = _WTOT if causal else 8 * 1024

    with ExitStack() as ctx:
        tc = ctx.enter_context(tile.TileContext(nc))
        consts = ctx.enter_context(tc.tile_pool(name="consts", bufs=1))
        qtkt = ctx.enter_context(tc.tile_pool(name="qtkt", bufs=1))
        v2p = ctx.enter_context(tc.tile_pool(name="v2", bufs=1))
        outp = ctx.enter_context(tc.tile_pool(name="outp", bufs=1))
        wop = ctx.enter_context(tc.tile_pool(name="wop", bufs=1))
        dramp = ctx.enter_context(tc.tile_pool(name="dram", bufs=1, space="DRAM"))

        ident = consts.tile([P, P], bf16, tag="ident", name="ident")
        nc.sync.dma_start(out=ident, in_=ident_d)
        ones_all = consts.tile([P, P], bf16, tag="ones", name="ones")
        nc.vector.memset(ones_all, 1.0)
        bias_sb = consts.tile([1, 3 * F], bf16, tag="bias", name="bias")
        nc.sync.dma_start(out=bias_sb, in_=bias_d.rearrange("a b -> (a b)").unsqueeze(0))
        wo_sb = wop.tile([P, 4, F], bf16, tag="wo", name="wo")
        nc.sync.dma_start(out=wo_sb, in_=wo_d.rearrange("(c p) f -> p c f", p=P))

        QT, KT, OT, V2 = {}, {}, {}, {}
        for pr in range(4):  # head pairs
            QT[pr] = qtkt.tile([P, S], bf16, tag=f"qt{pr}", name=f"qt{pr}")
            KT[pr] = qtkt.tile([P, S], bf16, tag=f"kt{pr}", name=f"kt{pr}")
            OT[pr] = outp.tile([P, S], bf16, tag=f"ot{pr}", name=f"ot{pr}")
        for hp in range(HG):
            V2[hp] = v2p.tile([P, 8, P], bf16, tag=f"v{hp}", name=f"v{hp}")

        # ---------- phase 1: projections + layout shuffles ----------
        with tc.tile_pool(name="xt", bufs=1) as xtp, \
                tc.tile_pool(name="wpool", bufs=1) as wp, \
                tc.tile_pool(name="qkvc", bufs=1) as qkvcp, \
                tc.tile_pool(name="pj", bufs=2, space="PSUM") as ppool, \
                tc.tile_pool(name="tp", bufs=2, space="PSUM") as tpool:
            xt = xtp.tile([P, 8, 4, P], bf16, tag="xt", name="xt")
            nc.sync.dma_start(
                out=xt, in_=xT_d.rearrange("(kc p) (rb r) -> p kc rb r", p=P, rb=4))

            qkvc = {}
            for t_i, w_d in enumerate([wq_d, wk_d, wv_d]):
                wt = []
                for i in range(8):
                    w_tile = wp.tile([P, F], bf16, tag=f"w{i}", name=f"w{i}")
                    nc.sync.dma_start(out=w_tile, in_=w_d[i * P:(i + 1) * P, :])
                    wt.append(w_tile)
                cc = qkvcp.tile([P, 4, F], bf16, tag=f"c{t_i}", name=f"c{t_i}")
                qkvc[t_i] = cc
                for rb in range(4):
                    for h2 in range(2):
                        ps = ppool.tile([P, 512], f32, tag="pj", name="pj")
                        for kc in range(8):
                            nc.tensor.matmul(
                                ps, xt[:, kc, rb, :],
                                wt[kc][:, 512 * h2:512 * (h2 + 1)],
                                start=(kc == 0), stop=False)
                        nc.tensor.matmul(
                            ps, ones_all[0:1, :],
                            bias_sb[0:1, 1024 * t_i + 512 * h2:1024 * t_i + 512 * h2 + 512],
                            start=False, stop=True)
                        nc.vector.tensor_copy(cc[:, rb, 512 * h2:512 * (h2 + 1)], ps)

            # V shuffle through DRAM into [k'-chunk partitions, d] layout (+ones)
            vs = dramp.tile([RW, F], bf16, tag="vs", name="vs")
            nc.sync.dma_start(out=vs.rearrange("(rb p) c -> p rb c", p=P),
                              in_=qkvc[2])
            for hp in range(HG):
                e = hp % 2
                nc.vector.memset(V2[hp], 0.0)
                dcol = 64 * e
                ones_col = 64 * (1 - e)
                src = vs[64 * hp:64 * hp + 64, :].rearrange(
                    "(j r) (cb d) -> (r cb) j d", j=8, cb=16)
                nc.sync.dma_start(out=V2[hp][:, :, dcol:dcol + 64], in_=src)
                nc.vector.memset(V2[hp][:, :, ones_col:ones_col + 1], 1.0)

            # Q^T / K^T via 64x64 PE transposes
            for t_i, dstmap in ((0, QT), (1, KT)):
                for hp in range(HG):
                    pr, e = hp // 2, hp % 2
                    base = 64 * e
                    src_rows = qkvc[t_i][base:base + 64, pr, :]
                    for half in range(2):
                        pst = tpool.tile([64, 512], bf16, tag="tp", name="tp")
                        for cb8 in range(8):
                            cb = 8 * half + cb8
                            nc.tensor.transpose(
                                pst[0:64, 64 * cb8:64 * cb8 + 64],
                                src_rows.rearrange("p (cb d) -> p cb d", cb=16)[:, cb, :],
                                ident[base:base + 64, base:base + 64])
                        dst = dstmap[pr][base:base + 64, :].rearrange(
                            "p (r cb) -> p r cb", cb=16)[:, :, 8 * half:8 * half + 8]
                        nc.vector.tensor_copy(
                            dst, pst[0:64, :].rearrange("p (cb8 r) -> p r cb8", cb8=8))

        # ---------- phase 2: attention, two rounds of 4 heads ----------
        for rnd in range(2):
            hps = list(range(4 * rnd, 4 * rnd + 4))
            with tc.tile_pool(name=f"em{rnd}", bufs=1) as emp, \
                    tc.tile_pool(name=f"rbc{rnd}", bufs=1) as rbcp, \
                    tc.tile_pool(name=f"m{rnd}", bufs=1) as miscp:
                eM, r1bc, r1raw, r1sb = {}, {}, {}, {}
                # 2a: packed str exp + softmax-1 denominators
                with tc.tile_pool(name=f"r1ps{rnd}", bufs=1, space="PSUM") as r1psp:
                    for hp in hps:
                        t = emp.tile([P, wtot], bf16, tag=f"e{hp % 4}", name=f"e{hp % 4}")
                        eM[hp] = t
                        for j in range(8):
                            jl = 128 * j if causal else 0
                            w = S - jl
                            nc.sync.dma_start(
                                out=t[:, woff[j]:woff[j] + w],
                                in_=str_d[hp, 128 * j:128 * (j + 1), jl:])
                        nc.scalar.activation(t, t, Exp)
                        ps_r1 = r1psp.tile([1, S], f32, tag="r1", name="r1")
                        for j in range(8):
                            jl = 128 * j if causal else 0
                            for h2 in range(2):
                                lo = max(512 * h2, jl)
                                hi = 512 * (h2 + 1)
                                if lo < hi:
                                    last_j = (3 if h2 == 0 else 7) if causal else 7
                                    nc.tensor.matmul(
                                        ps_r1[0:1, lo:hi],
                                        ones_all[:, 0:1],
                                        eM[hp][:, woff[j] + lo - jl:woff[j] + hi - jl],
                                        start=(j == 0), stop=(j == last_j))
                        rr = miscp.tile([1, S], f32, tag=f"rr{hp % 4}", name=f"rr{hp % 4}")
                        r1raw[hp] = rr
                        nc.vector.tensor_copy(rr, ps_r1)
                # 2b: batched reciprocals (one Ln block, one Exp block)
                for hp in hps:
                    nc.scalar.activation(r1raw[hp], r1raw[hp], Ln)
                for hp in hps:
                    rs = miscp.tile([1, S], bf16, tag=f"rs{hp % 4}", name=f"rs{hp % 4}")
                    r1sb[hp] = rs
                    nc.scalar.activation(rs, r1raw[hp], Exp, scale=-1.0)
                # 2c: broadcast 1/R1 across partitions
                with tc.tile_pool(name=f"bc{rnd}", bufs=2, space="PSUM") as bcp:
                    for hp in hps:
                        rbc = rbcp.tile([P, S], bf16, tag=f"rb{hp % 4}", name=f"rb{hp % 4}")
                        r1bc[hp] = rbc
                        for h2 in range(2):
                            sl = slice(512 * h2, 512 * (h2 + 1))
                            psb = bcp.tile([P, 512], f32, tag="bc", name="bc")
                            nc.tensor.matmul(psb, ones_all[0:1, :], r1sb[hp][0:1, sl],
                                             start=True, stop=True)
                            nc.vector.tensor_copy(rbc[:, sl], psb)
                # 2d/2e: attention + PV + softmax-2 normalize
                with tc.tile_pool(name=f"ep{rnd}", bufs=3) as epool, \
                        tc.tile_pool(name=f"qk{rnd}", bufs=2, space="PSUM") as qkps, \
                        tc.tile_pool(name=f"pv{rnd}", bufs=1, space="PSUM") as pvps:
                    for hp in hps:
                        pr, e = hp // 2, hp % 2
                        base = 64 * e
                        pv = pvps.tile([P, S], f32, tag="pv", name="pv")
                        for j in range(8):
                            jl = 128 * j if causal else 0
                            Ej = epool.tile([P, S], bf16, tag="E", name="E")
                            qk = qkps.tile([P, S], f32, tag="qk", name="qk")
                            for h2 in range(2):
                                nc.tensor.matmul(
                                    qk[:, 512 * h2:512 * (h2 + 1)],
                                    KT[pr][base:base + 64, 128 * j:128 * (j + 1)],
                                    QT[pr][base:base + 64, 512 * h2:512 * (h2 + 1)],
                                    start=True, stop=True)
                            # add sm into the qk PSUM so ONE full-width exp
                            # covers both the masked and live regions
                            if jl < S:
                                nc.vector.tensor_mul(
                                    Ej[:, jl:], eM[hp][:, woff[j]:woff[j] + S - jl],
                                    r1bc[hp][:, jl:])
                                nc.vector.tensor_add(qk[:, jl:], qk[:, jl:], Ej[:, jl:])
                            nc.scalar.activation(Ej, qk, Exp)
                            for h2 in range(2):
                                nc.tensor.matmul(
                                    pv[:, 512 * h2:512 * (h2 + 1)],
                                    V2[hp][:, j, :],
                                    Ej[:, 512 * h2:512 * (h2 + 1)],
                                    start=(j == 0), stop=(j == 7))
                        # normalize rows of PV by 1/rowsum2 (ones column)
                        sum_row = 64 * (1 - e)
                        dlo = 64 * e
                        r2f = miscp.tile([P, S], f32, tag="r2f", name="r2f")
                        r2sb = miscp.tile([P, S], bf16, tag="r2sb", name="r2sb")
                        r2bc = miscp.tile([P, S], bf16, tag="r2bc", name="r2bc")
                        nc.scalar.activation(r2f[sum_row:sum_row + 1, :],
                                             pv[sum_row:sum_row + 1, :], Ln)
                        nc.scalar.activation(r2sb[sum_row:sum_row + 1, :],
                                             r2f[sum_row:sum_row + 1, :], Exp, scale=-1.0)
                        with tc.tile_pool(name=f"b2{rnd}{hp % 4}", bufs=1,
                                          space="PSUM") as b2p:
                            for h2 in range(2):
                                sl = slice(512 * h2, 512 * (h2 + 1))
                                psb = b2p.tile([P, 512], f32, tag="b2", name="b2")
                                nc.tensor.matmul(psb[dlo:dlo + 64, :],
                                                 ones_all[sum_row:sum_row + 1, 0:64],
                                                 r2sb[sum_row:sum_row + 1, sl],
                                                 start=True, stop=True)
                                nc.vector.tensor_copy(r2bc[dlo:dlo + 64, sl],
                                                      psb[dlo:dlo + 64, :])
                                nc.vector.tensor_mul(OT[pr][dlo:dlo + 64, sl],
                                                     pv[dlo:dlo + 64, sl],
                                                     r2bc[dlo:dlo + 64, sl])

        # ---------- phase 3: partial output projection ----------
        with tc.tile_pool(name="os", bufs=3) as osp, \
                tc.tile_pool(name="ops", bufs=2, space="PSUM") as opsum:
            for fo in range(8):
                ot = osp.tile([P, S], f32, tag="os", name="os")
                for h2 in range(2):
                    ps = opsum.tile([P, 512], f32, tag="op", name="op")
                    for pr in range(4):
                        nc.tensor.matmul(ps, wo_sb[:, pr, 128 * fo:128 * (fo + 1)],
                                         OT[pr][:, 512 * h2:512 * (h2 + 1)],
                                         start=(pr == 0), stop=(pr == 3))
                    nc.vector.tensor_copy(ot[:, 512 * h2:512 * (h2 + 1)], ps)
                nc.sync.dma_start(out=out_d[128 * fo:128 * (fo + 1), :], in_=ot)

    nc.compile()
    return nc


def _prep_host(x, str_mat, attn_mask, Wq, bq, Wk, bk, Wv, bv, Wo, bo):
    bf = _np_bf16()
    x = np.asarray(x, np.float32)
    str_mat = np.asarray(str_mat, np.float32)
    attn_mask = np.asarray(attn_mask, np.float32)
    mask = attn_mask[:, 0]  # [b, s, s]
    causal = bool((mask == np.tril(np.ones((S, S), np.float32))[None]).all())
    strT = np.where(mask[:, None] == 0.0, NEG_FILL, str_mat).transpose(0, 1, 3, 2)
    strT = strT.astype(bf)
    xT = x.transpose(0, 2, 1).astype(bf)  # [b, f, s]
    Wq_s = (np.asarray(Wq, np.float32) / D).astype(bf)
    bq_s = (np.asarray(bq, np.float32) / D)
    bias = np.stack([bq_s, np.asarray(bk, np.float32),
                     np.asarray(bv, np.float32)]).astype(bf)
    Wk_c = np.asarray(Wk, np.float32).astype(bf)
    Wv_c = np.asarray(Wv, np.float32).astype(bf)
    Wo_c = np.asarray(Wo, np.float32).astype(bf)
    ident = np.eye(P, dtype=np.float32).astype(bf)
    in_maps = []
    for c in range(NCORES):
        bc, g = c // 2, c % 2
        in_maps.append({
            "xT": np.ascontiguousarray(xT[bc, :, RW * g:RW * (g + 1)]),
            "strT": np.ascontiguousarray(strT[bc, HG * g:HG * (g + 1)]),
            "wq": Wq_s, "wk": Wk_c, "wv": Wv_c,
            "wo": np.ascontiguousarray(Wo_c[RW * g:RW * (g + 1)]),
            "bqkv": bias, "ident": ident,
        })
    return in_maps, causal


def kernel(**inputs):
    from concourse.bass_utils import run_bass_kernel_spmd

    in_maps, causal = _prep_host(**inputs)
    key = ("v2", causal)
    if key not in _CACHE:
        _CACHE[key] = _build_nc(causal=causal)
    nc = _CACHE[key]
    res = run_bass_kernel_spmd(nc, in_maps, core_ids=list(range(NCORES)))
    partials = [r["outT"] for r in res.results]
    out = np.stack([partials[2 * bc] + partials[2 * bc + 1] for bc in range(B)])
    out = out.transpose(0, 2, 1) + np.asarray(bo_arr(inputs), np.float32)
    return np.ascontiguousarray(out.astype(np.float32))


def bo_arr(inputs):
    return np.asarray(inputs["bo"], np.float32)


# revision 18
# speedup vs baseline: 2.1072x; 1.0454x over previous
"""Trainium2 Bass kernel for nn_MHInrAttn (sparse_attention, b=4 s=1024 f=1024 h=16).

v2 sharding (8 NeuronCores): core c -> (batch c//2, head-group c%2).
Each core: 8 heads of one batch. The reference uses a raw .reshape with NO
transpose, so head h's Q/K/V come from rows [64h, 64h+64) of the projected
[s, f] matrix -> a head-group needs only x rows [512g, 512g+512).

Per core: project Q/K/V for 512 rows, attention for 8 heads in scores^T
[k, q] orientation, partial output projection through its 512 Wo rows.
Host: sum partial pairs per batch, transpose, add bo.

Device details:
  - all matmul operands bf16 (1 cyc/row on PE), fp32 PSUM accumulation
  - str^T streams causally trimmed into a PACKED eM tile (one wide exp/head)
  - softmax-1 k-sums via ones-column PE matmul; 1/x via Ln->Exp on ACT with
    Ln/Exp batched per phase (they live in different ACT table sets)
  - qk pair tile [128, 1024] spans 2 PSUM banks -> wide ACT/DVE ops
  - PV carries a ones column in V for softmax-2 row sums for free
"""

import numpy as np

B, S, F, H, D = 4, 1024, 1024, 16, 64
NCORES = 8
HG = 8           # heads per core (head-group)
RW = 512         # proj rows per core
P = 128
NEG_FILL = -40.0

_CACHE = {}

_WOFF = [0, 1024, 1920, 2688, 3328, 3840, 4224, 4480]  # packed eM offsets
_WTOT = 4608


def _np_bf16():
    import ml_dtypes

    return ml_dtypes.bfloat16


def _build_nc(causal=True):
    from contextlib import ExitStack

    import concourse.bacc as bacc
    import concourse.tile as tile
    from concourse import mybir

    dt = mybir.dt
    f32 = dt.float32
    bf16 = dt.bfloat16
    Exp = mybir.ActivationFunctionType.Exp
    Ln = mybir.ActivationFunctionType.Ln

    nc = bacc.Bacc("TRN2", target_bir_lowering=False, debug=False)

    xT_d = nc.dram_tensor("xT", [F, RW], bf16, kind="ExternalInput").ap()
    str_d = nc.dram_tensor("strT", [HG, S, S], bf16, kind="ExternalInput").ap()
    wq_d = nc.dram_tensor("wq", [F, F], bf16, kind="ExternalInput").ap()
    wk_d = nc.dram_tensor("wk", [F, F], bf16, kind="ExternalInput").ap()
    wv_d = nc.dram_tensor("wv", [F, F], bf16, kind="ExternalInput").ap()
    wo_d = nc.dram_tensor("wo", [RW, F], bf16, kind="ExternalInput").ap()
    bias_d = nc.dram_tensor("bqkv", [3, F], bf16, kind="ExternalInput").ap()
    ident_d = nc.dram_tensor("ident", [P, P], bf16, kind="ExternalInput").ap()
    out_d = nc.dram_tensor("outT", [F, S], f32, kind="ExternalOutput").ap()

    woff = _WOFF if causal else [1024 * j for j in range(8)]
    wtot = _WTOT if causal else 8 * 1024

    with ExitStack() as ctx:
        tc = ctx.enter_context(tile.TileContext(nc))
        consts = ctx.enter_context(tc.tile_pool(name="consts", bufs=1))
        qtkt = ctx.enter_context(tc.tile_pool(name="qtkt", bufs=1))
        v2p = ctx.enter_context(tc.tile_pool(name="v2", bufs=1))
        outp = ctx.enter_context(tc.tile_pool(name="outp", bufs=1))
        wop = ctx.enter_context(tc.tile_pool(name="wop", bufs=1))
        dramp = ctx.enter_context(tc.tile_pool(name="dram", bufs=1, space="DRAM"))

        ident = consts.tile([P, P], bf16, tag="ident", name="ident")
        nc.sync.dma_start(out=ident, in_=ident_d)
        ones_all = consts.tile([P, P], bf16, tag="ones", name="ones")
        nc.vector.memset(ones_all, 1.0)
        bias_sb = consts.tile([1, 3 * F], bf16, tag="bias", name="bias")
        nc.sync.dma_start(out=bias_sb, in_=bias_d.rearrange("a b -> (a b)").unsqueeze(0))
        wo_sb = wop.tile([P, 4, F], bf16, tag="wo", name="wo")
        nc.sync.dma_start(out=wo_sb, in_=wo_d.rearrange("(c p) f -> p c f", p=P))

        QT, KT, OT, V2 = {}, {}, {}, {}
        for pr in range(4):  # head pairs
            QT[pr] = qtkt.tile([P, S], bf16, tag=f"qt{pr}", name=f"qt{pr}")
            KT[pr] = qtkt.tile([P, S], bf16, tag=f"kt{pr}", name=f"kt{pr}")
            OT[pr] = outp.tile([P, S], bf16, tag=f"ot{pr}", name=f"ot{pr}")
        for hp in range(HG):
            V2[hp] = v2p.tile([P, 8, P], bf16, tag=f"v{hp}", name=f"v{hp}")

        # ---------- phase 1: projections + layout shuffles ----------
        with tc.tile_pool(name="xt", bufs=1) as xtp, \
                tc.tile_pool(name="wpool", bufs=1) as wp, \
                tc.tile_pool(name="qkvc", bufs=1) as qkvcp, \
                tc.tile_pool(name="pj", bufs=2, space="PSUM") as ppool, \
                tc.tile_pool(name="tp", bufs=2, space="PSUM") as tpool:
            xt = xtp.tile([P, 8, 4, P], bf16, tag="xt", name="xt")
            nc.sync.dma_start(
                out=xt, in_=xT_d.rearrange("(kc p) (rb r) -> p kc rb r", p=P, rb=4))

            qkvc = {}
            for t_i, w_d in enumerate([wq_d, wk_d, wv_d]):
                wt = []
                for i in range(8):
                    w_tile = wp.tile([P, F], bf16, tag=f"w{i}", name=f"w{i}")
                    nc.sync.dma_start(out=w_tile, in_=w_d[i * P:(i + 1) * P, :])
                    wt.append(w_tile)
                cc = qkvcp.tile([P, 4, F], bf16, tag=f"c{t_i}", name=f"c{t_i}")
                qkvc[t_i] = cc
                for rb in range(4):
                    for h2 in range(2):
                        ps = ppool.tile([P, 512], f32, tag="pj", name="pj")
                        for kc in range(8):
                            nc.tensor.matmul(
                                ps, xt[:, kc, rb, :],
                                wt[kc][:, 512 * h2:512 * (h2 + 1)],
                                start=(kc == 0), stop=False)
                        nc.tensor.matmul(
                            ps, ones_all[0:1, :],
                            bias_sb[0:1, 1024 * t_i + 512 * h2:1024 * t_i + 512 * h2 + 512],
                            start=False, stop=True)
                        nc.vector.tensor_copy(cc[:, rb, 512 * h2:512 * (h2 + 1)], ps)

            # V shuffle through DRAM into [k'-chunk partitions, d] layout (+ones)
            vs = dramp.tile([RW, F], bf16, tag="vs", name="vs")
            nc.sync.dma_start(out=vs.rearrange("(rb p) c -> p rb c", p=P),
                              in_=qkvc[2])
            for hp in range(HG):
                e = hp % 2
                nc.vector.memset(V2[hp], 0.0)
                dcol = 64 * e
                ones_col = 64 * (1 - e)
                src = vs[64 * hp:64 * hp + 64, :].rearrange(
                    "(j r) (cb d) -> (r cb) j d", j=8, cb=16)
                nc.sync.dma_start(out=V2[hp][:, :, dcol:dcol + 64], in_=src)
                nc.vector.memset(V2[hp][:, :, ones_col:ones_col + 1], 1.0)

            # Q^T / K^T via 64x64 PE transposes
            for t_i, dstmap in ((0, QT), (1, KT)):
                for hp in range(HG):
                    pr, e = hp // 2, hp % 2
                    base = 64 * e
                    src_rows = qkvc[t_i][base:base + 64, pr, :]
                    for half in range(2):
                        pst = tpool.tile([64, 512], bf16, tag="tp", name="tp")
                        for cb8 in range(8):
                            cb = 8 * half + cb8
                            nc.tensor.transpose(
                                pst[0:64, 64 * cb8:64 * cb8 + 64],
                                src_rows.rearrange("p (cb d) -> p cb d", cb=16)[:, cb, :],
                                ident[base:base + 64, base:base + 64])
                        dst = dstmap[pr][base:base + 64, :].rearrange(
                            "p (r cb) -> p r cb", cb=16)[:, :, 8 * half:8 * half + 8]
                        nc.vector.tensor_copy(
                            dst, pst[0:64, :].rearrange("p (cb8 r) -> p r cb8", cb8=8))

        # ---------- phase 2: attention, two rounds of 4 heads ----------
        for rnd in range(2):
            hps = list(range(4 * rnd, 4 * rnd + 4))
            with tc.tile_pool(name=f"em{rnd}", bufs=1) as emp, \
                    tc.tile_pool(name=f"rbc{rnd}", bufs=1) as rbcp, \
                    tc.tile_pool(name=f"m{rnd}", bufs=1) as miscp:
                eM, r1bc, r1raw, r1sb = {}, {}, {}, {}
                # 2a: packed str exp + softmax-1 denominators
                with tc.tile_pool(name=f"r1ps{rnd}", bufs=1, space="PSUM") as r1psp:
                    for hp in hps:
                        t = emp.tile([P, wtot], bf16, tag=f"e{hp % 4}", name=f"e{hp % 4}")
                        eM[hp] = t
                        for j in range(8):
                            jl = 128 * j if causal else 0
                            w = S - jl
                            nc.sync.dma_start(
                                out=t[:, woff[j]:woff[j] + w],
                                in_=str_d[hp, 128 * j:128 * (j + 1), jl:])
                        nc.scalar.activation(t, t, Exp)
                        ps_r1 = r1psp.tile([1, S], f32, tag="r1", name="r1")
                        for j in range(8):
                            jl = 128 * j if causal else 0
                            for h2 in range(2):
                                lo = max(512 * h2, jl)
                                hi = 512 * (h2 + 1)
                                if lo < hi:
                                    last_j = (3 if h2 == 0 else 7) if causal else 7
                                    nc.tensor.matmul(
                                        ps_r1[0:1, lo:hi],
                                        ones_all[:, 0:1],
                                        eM[hp][:, woff[j] + lo - jl:woff[j] + hi - jl],
                                        start=(j == 0), stop=(j == last_j))
                        rr = miscp.tile([1, S], f32, tag=f"rr{hp % 4}", name=f"rr{hp % 4}")
                        r1raw[hp] = rr
                        nc.vector.tensor_copy(rr, ps_r1)
                # 2b: batched reciprocals (one Ln block, one Exp block)
                for hp in hps:
                    nc.scalar.activation(r1raw[hp], r1raw[hp], Ln)
                for hp in hps:
                    rs = miscp.tile([1, S], bf16, tag=f"rs{hp % 4}", name=f"rs{hp % 4}")
                    r1sb[hp] = rs
                    nc.scalar.activation(rs, r1raw[hp], Exp, scale=-1.0)
                # 2c: broadcast 1/R1 across partitions
                with tc.tile_pool(name=f"bc{rnd}", bufs=2, space="PSUM") as bcp:
                    for hp in hps:
                        rbc = rbcp.tile([P, S], bf16, tag=f"rb{hp % 4}", name=f"rb{hp % 4}")
                        r1bc[hp] = rbc
                        for h2 in range(2):
                            sl = slice(512 * h2, 512 * (h2 + 1))
                            psb = bcp.tile([P, 512], f32, tag="bc", name="bc")
                            nc.tensor.matmul(psb, ones_all[0:1, :], r1sb[hp][0:1, sl],
                                             start=True, stop=True)
                            nc.vector.tensor_copy(rbc[:, sl], psb)
                # 2d/2e: attention + PV + softmax-2 normalize
                with tc.tile_pool(name=f"ep{rnd}", bufs=3) as epool, \
                        tc.tile_pool(name=f"qk{rnd}", bufs=2, space="PSUM") as qkps, \
                        tc.tile_pool(name=f"pv{rnd}", bufs=2, space="PSUM") as pvps:
                    for hp in hps:
                        pr, e = hp // 2, hp % 2
                        base = 64 * e
                        pv = pvps.tile([P, S], f32, tag="pv", name="pv")
                        for j in range(8):
                            jl = 128 * j if causal else 0
                            Ej = epool.tile([P, S], bf16, tag="E", name="E")
                            qk = qkps.tile([P, S], f32, tag="qk", name="qk")
                            for h2 in range(2):
                                nc.tensor.matmul(
                                    qk[:, 512 * h2:512 * (h2 + 1)],
                                    KT[pr][base:base + 64, 128 * j:128 * (j + 1)],
                                    QT[pr][base:base + 64, 512 * h2:512 * (h2 + 1)],
                                    start=True, stop=True)
                            # add sm into the qk PSUM so ONE full-width exp
                            # covers both the masked and live regions
                            if jl < S:
                                nc.vector.tensor_mul(
                                    Ej[:, jl:], eM[hp][:, woff[j]:woff[j] + S - jl],
                                    r1bc[hp][:, jl:])
                                nc.vector.tensor_add(qk[:, jl:], qk[:, jl:], Ej[:, jl:])
                            nc.scalar.activation(Ej, qk, Exp)
                            for h2 in range(2):
                                nc.tensor.matmul(
                                    pv[:, 512 * h2:512 * (h2 + 1)],
                                    V2[hp][:, j, :],
                                    Ej[:, 512 * h2:512 * (h2 + 1)],
                                    start=(j == 0), stop=(j == 7))
                        # normalize rows of PV by 1/rowsum2 (ones column).
                        # DVE stages the sum row to partition 0 (cross-
                        # partition copies are fine on DVE), the reciprocal
                        # broadcast runs on the idle GPSIMD engine -- no PSUM
                        # bank needed, so pv can double-buffer
                        sum_row = 64 * (1 - e)
                        dlo = 64 * e
                        r2f = miscp.tile([1, S], f32, tag="r2f", name="r2f")
                        r2s = miscp.tile([1, S], bf16, tag="r2s", name="r2s")
                        r2b = miscp.tile([P, S], bf16, tag="r2b", name="r2b")
                        nc.vector.tensor_copy(r2f, pv[sum_row:sum_row + 1, :])
                        nc.scalar.activation(r2f, r2f, Ln)
                        nc.scalar.activation(r2s, r2f, Exp, scale=-1.0)
                        nc.gpsimd.partition_broadcast(r2b, r2s)
                        nc.vector.tensor_mul(OT[pr][dlo:dlo + 64, :],
                                             pv[dlo:dlo + 64, :],
                                             r2b[dlo:dlo + 64, :])

        # ---------- phase 3: partial output projection ----------
        with tc.tile_pool(name="os", bufs=3) as osp, \
                tc.tile_pool(name="ops", bufs=2, space="PSUM") as opsum:
            for fo in range(8):
                ot = osp.tile([P, S], f32, tag="os", name="os")
                for h2 in range(2):
                    ps = opsum.tile([P, 512], f32, tag="op", name="op")
                    for pr in range(4):
                        nc.tensor.matmul(ps, wo_sb[:, pr, 128 * fo:128 * (fo + 1)],
                                         OT[pr][:, 512 * h2:512 * (h2 + 1)],
                                         start=(pr == 0), stop=(pr == 3))
                    nc.vector.tensor_copy(ot[:, 512 * h2:512 * (h2 + 1)], ps)
                nc.sync.dma_start(out=out_d[128 * fo:128 * (fo + 1), :], in_=ot)

    nc.compile()
    return nc


def _prep_host(x, str_mat, attn_mask, Wq, bq, Wk, bk, Wv, bv, Wo, bo):
    bf = _np_bf16()
    x = np.asarray(x, np.float32)
    str_mat = np.asarray(str_mat, np.float32)
    attn_mask = np.asarray(attn_mask, np.float32)
    mask = attn_mask[:, 0]  # [b, s, s]
    causal = bool((mask == np.tril(np.ones((S, S), np.float32))[None]).all())
    strT = np.where(mask[:, None] == 0.0, NEG_FILL, str_mat).transpose(0, 1, 3, 2)
    strT = strT.astype(bf)
    xT = x.transpose(0, 2, 1).astype(bf)  # [b, f, s]
    Wq_s = (np.asarray(Wq, np.float32) / D).astype(bf)
    bq_s = (np.asarray(bq, np.float32) / D)
    bias = np.stack([bq_s, np.asarray(bk, np.float32),
                     np.asarray(bv, np.float32)]).astype(bf)
    Wk_c = np.asarray(Wk, np.float32).astype(bf)
    Wv_c = np.asarray(Wv, np.float32).astype(bf)
    Wo_c = np.asarray(Wo, np.float32).astype(bf)
    ident = np.eye(P, dtype=np.float32).astype(bf)
    in_maps = []
    for c in range(NCORES):
        bc, g = c // 2, c % 2
        in_maps.append({
            "xT": np.ascontiguousarray(xT[bc, :, RW * g:RW * (g + 1)]),
            "strT": np.ascontiguousarray(strT[bc, HG * g:HG * (g + 1)]),
            "wq": Wq_s, "wk": Wk_c, "wv": Wv_c,
            "wo": np.ascontiguousarray(Wo_c[RW * g:RW * (g + 1)]),
            "bqkv": bias, "ident": ident,
        })
    return in_maps, causal


def kernel(**inputs):
    from concourse.bass_utils import run_bass_kernel_spmd

    in_maps, causal = _prep_host(**inputs)
    key = ("v2", causal)
    if key not in _CACHE:
        _CACHE[key] = _build_nc(causal=causal)
    nc = _CACHE[key]
    res = run_bass_kernel_spmd(nc, in_maps, core_ids=list(range(NCORES)))
    partials = [r["outT"] for r in res.results]
    out = np.stack([partials[2 * bc] + partials[2 * bc + 1] for bc in range(B)])
    out = out.transpose(0, 2, 1) + np.asarray(bo_arr(inputs), np.float32)
    return np.ascontiguousarray(out.astype(np.float32))


def bo_arr(inputs):
    return np.asarray(inputs["bo"], np.float32)
